# revision 16
# baseline (speedup 1.0000x reference)
"""Trainium2 Bass kernel for the EntropyResidualBlock (two masked 5x5 convs,
PReLU, residual) on 8 NeuronCores.

Sharding: 8 cores = 2 batches x 4 H-strips of 64 rows. Each core recomputes a
2-row y1 halo at the top of its strip (no cross-core communication); x halos
are host-zero-padded and a per-core halo-mask input zeroes the y1 halo rows
for strips at the image top (so conv2 sees correct zero padding).

The PixelCNN mask leaves only 13 of 25 taps nonzero, so each conv row is 13
taps x 3 ci-chunks x 3 co-chunks of [128,128]x[128,512] matmuls accumulated
in PSUM. To beat the bf16 PE roofline, 5 taps' (cic0,cic1) chunk pairs run as
fp8e4 DoubleRow matmuls (K=256 in one 512-cycle pass, measured 2.0x); the
rest stay bf16. Scales: w8 = e4m3(w*1024), x8/y18 = e4m3(16*act), bf16
weights *16384 so every matmul accumulates 16384*true in PSUM; the PReLU
epilogue applies scale=1/16384. Error budget (vs 2e-2 gate): ~1.9e-2,
verified against the exact reference inputs in simulation.

conv1 -> conv2 are fused through rolling 4-row SBUF ring buffers (bf16 + fp8
copies of each activation row).
"""

import os
import sys

import numpy as np
import ml_dtypes

for _p in ("/opt/trn_rl_repo",):
    if os.path.isdir(_p) and _p not in sys.path:
        sys.path.append(_p)

import concourse.bass as bass  # noqa: E402
import concourse.tile as tile  # noqa: E402
from concourse import bacc, mybir  # noqa: E402
from concourse.bass import ds  # noqa: E402
from concourse.bass_utils import run_bass_kernel_spmd  # noqa: E402

BF16NP = ml_dtypes.bfloat16
E4NP = ml_dtypes.float8_e4m3
F32 = mybir.dt.float32
BF16 = mybir.dt.bfloat16
FP8 = mybir.dt.float8e4
AF = mybir.ActivationFunctionType
DRM = mybir.MatmulPerfMode.DoubleRow

B, C, H, W = 2, 384, 256, 512
NG, CPN, KS, PAD = 16, 24, 5, 2
NCORES = 8
SPB = 4            # strips per batch
HS = H // SPB      # 64 output rows per core
WP = 520           # padded row width in SBUF/DRAM (2 left pad + 512 + 6)
WP8 = 528          # fp8 ring row pitch (mult of 16 for DoubleRow pair step)
NR = HS + 5        # x rows staged per core: r0-4 .. r0+64 (last is prefetch slack)
TAPS = [(kh, kw) for kh in (0, 1) for kw in range(KS)] + [(2, 0), (2, 1), (2, 2)]
NT = len(TAPS)     # 13
FP8_TAPS = [(0, 0), (0, 2), (0, 4), (1, 1), (1, 3)]
NF = len(FP8_TAPS)
SW, SX = 1024.0, 16.0      # fp8 scales (pow2; max|w|*SW~111<240, 16*act<240)
SB = SW * SX               # bf16 weight scale; PSUM holds SB * true


def _build_mask() -> np.ndarray:
    m = np.zeros((C, C, KS, KS), np.float32)
    m[:, :, :PAD, :] = 1.0
    m[:, :, PAD, :PAD] = 1.0
    g = np.arange(C) // CPN
    m[:, :, PAD, PAD] = (g[None, :] <= g[:, None]).astype(np.float32)
    return m


def _build_nc():
    nc = bacc.Bacc("TRN2", target_bir_lowering=False, debug=False,
                   num_devices=NCORES)
    xs_d = nc.dram_tensor("xs", [128, NR * 3, WP], BF16, kind="ExternalInput").ap()
    w1_d = nc.dram_tensor("w1t", [128, 3, NT, 3, 128], BF16,
                          kind="ExternalInput").ap()
    w2_d = nc.dram_tensor("w2t", [128, 3, NT, 3, 128], BF16,
                          kind="ExternalInput").ap()
    w18_d = nc.dram_tensor("w18", [128, 3, NF, 2, 128], FP8,
                           kind="ExternalInput").ap()
    w28_d = nc.dram_tensor("w28", [128, 3, NF, 2, 128], FP8,
                           kind="ExternalInput").ap()
    b1_d = nc.dram_tensor("b1c", [128, 3], F32, kind="ExternalInput").ap()
    a1_d = nc.dram_tensor("a1c", [128, 3], F32, kind="ExternalInput").ap()
    b2_d = nc.dram_tensor("b2c", [128, 3], F32, kind="ExternalInput").ap()
    a2_d = nc.dram_tensor("a2c", [128, 3], F32, kind="ExternalInput").ap()
    b116_d = nc.dram_tensor("b1c16", [128, 3], F32, kind="ExternalInput").ap()
    hm_d = nc.dram_tensor("hm", [128, 2], F32, kind="ExternalInput").ap()
    # rows 0,1 are dummies (garbage from the 2-row store lag); host reads 2..65
    ys_d = nc.dram_tensor("ys", [128, (HS + 2) * 3, W], BF16,
                          kind="ExternalOutput").ap()

    with tile.TileContext(nc) as tc:
        with tc.tile_pool(name="wp", bufs=1) as wp, \
             tc.tile_pool(name="cp", bufs=1) as cp, \
             tc.tile_pool(name="ring", bufs=1) as rp, \
             tc.tile_pool(name="op", bufs=4) as op, \
             tc.tile_pool(name="pp", bufs=8, space="PSUM") as pp:

            w18 = wp.tile([128, 3, NF, 2, 128], FP8, name="w18sb", tag="w18sb")
            nc.gpsimd.dma_start(out=w18, in_=w18_d)
            w1t = wp.tile([128, 3, NT, 3, 128], BF16, name="w1sb", tag="w1sb")
            # coc0 arrives per-tap so the very first matmuls aren't gated on
            # a bulk transfer; later chunks can be coarse
            for _t in range(NT):
                nc.gpsimd.dma_start(out=w1t[:, 0, _t], in_=w1_d[:, 0, _t])
            for _c in (1, 2):
                nc.gpsimd.dma_start(out=w1t[:, _c], in_=w1_d[:, _c])
            b1c = cp.tile([128, 3], F32, name="b1sb", tag="b1sb")
            nc.gpsimd.dma_start(out=b1c, in_=b1_d)
            a1c = cp.tile([128, 3], F32, name="a1sb", tag="a1sb")
            nc.gpsimd.dma_start(out=a1c, in_=a1_d)
            b2c = cp.tile([128, 3], F32, name="b2sb", tag="b2sb")
            nc.gpsimd.dma_start(out=b2c, in_=b2_d)
            a2c = cp.tile([128, 3], F32, name="a2sb", tag="a2sb")
            nc.gpsimd.dma_start(out=a2c, in_=a2_d)
            b1c16 = cp.tile([128, 3], F32, name="b116sb", tag="b116sb")
            nc.gpsimd.dma_start(out=b1c16, in_=b116_d)
            hm = cp.tile([128, 2], F32, name="hmsb", tag="hmsb")
            nc.gpsimd.dma_start(out=hm, in_=hm_d)

            xb = [rp.tile([128, 3, WP], BF16, name=f"xb{j}", tag=f"xb{j}")
                  for j in range(4)]
            x8 = [rp.tile([128, 2, WP8], FP8, name=f"x8{j}", tag=f"x8{j}")
                  for j in range(4)]
            y1 = [rp.tile([128, 3, WP], BF16, name=f"y1r{j}", tag=f"y1r{j}")
                  for j in range(4)]
            y18 = [rp.tile([128, 2, WP8], FP8, name=f"y18r{j}", tag=f"y18r{j}")
                   for j in range(4)]
            for j in range(4):
                # W-pad columns of the conv2-input rings stay zero forever
                nc.vector.memset(y1[j][:, :, 0:PAD], 0.0)
                nc.vector.memset(y1[j][:, :, PAD + W:WP], 0.0)
                nc.vector.memset(y18[j][:, :, 0:PAD], 0.0)
                nc.vector.memset(y18[j][:, :, PAD + W:WP8], 0.0)

            def load_x_row(row_expr, slot, eng=None):
                # one DMA: [128, 3, WP] bf16 row (host zero-padded), + fp8 cast
                (eng or nc.sync).dma_start(out=xb[slot],
                                           in_=xs_d[:, ds(row_expr * 3, 3), :])
                nc.scalar.activation(x8[slot][:, :, 0:WP], xb[slot][:, 0:2, :],
                                     AF.Copy, scale=SX)

            def conv_row(wt, w8t, ring, ring8, hmod, epilogue, dh0_last=False):
                taps = sorted(TAPS, key=lambda t: t[0] == 2) if dh0_last else TAPS
                for coc in range(3):
                    ps = pp.tile([128, W], F32, name="ps", tag="ps")
                    # fp8 DoubleRow block first (mode switches only at block
                    # edges), then the bf16 block
                    for n8, (kh, kw) in enumerate(FP8_TAPS):
                        dh, dw = kh - 2, kw - 2
                        src8 = ring8[(hmod + dh) % 4]
                        fpi = FP8_TAPS.index((kh, kw))
                        nc.tensor.matmul(
                            ps,
                            lhsT=w8t[:, coc, fpi],
                            rhs=src8[:, 0:2, PAD + dw: PAD + dw + W],
                            start=(n8 == 0), stop=False, perf_mode=DRM)
                    work = [(kh, kw, cic) for (kh, kw) in taps for cic in range(3)
                            # fp8 pairs covered above; group-causal center tap:
                            # ci chunk 2 never feeds co chunk 0 - weights all 0
                            if not ((kh, kw) in FP8_TAPS and cic < 2)
                            and not (kh == 2 and kw == 2 and cic == 2 and coc == 0)]
                    for n, (kh, kw, cic) in enumerate(work):
                        dh, dw = kh - 2, kw - 2
                        src = ring[(hmod + dh) % 4]
                        ti = TAPS.index((kh, kw))
                        nc.tensor.matmul(
                            ps,
                            lhsT=wt[:, coc, ti, cic, :],
                            rhs=src[:, cic, PAD + dw: PAD + dw + W],
                            start=False, stop=(n == len(work) - 1))
                    epilogue(coc, ps)

            def y1_epilogue(slot):
                def f(coc, ps):
                    nc.scalar.activation(
                        y1[slot][:, coc, PAD:PAD + W], ps, AF.Prelu,
                        bias=b1c[:, coc:coc + 1], scale=1.0 / SB,
                        alpha=a1c[:, coc:coc + 1])
                    if coc < 2:
                        # fp8 copy of y1 (x16) for conv2's DoubleRow taps
                        nc.scalar.activation(
                            y18[slot][:, coc, PAD:PAD + W], ps, AF.Prelu,
                            bias=b1c16[:, coc:coc + 1], scale=SX / SB,
                            alpha=a1c[:, coc:coc + 1])
                return f

            # ---- prologue: x[-4..-1], then y1[-2], y1[-1] (halo, maskable) ----
            for j in range(4):
                # spread the first loads across the two DMA-capable queues so
                # their rings start in parallel, shortening the pipeline head
                load_x_row(j, j, eng=(nc.sync, nc.scalar, nc.sync,
                                      nc.scalar)[j])
            w28 = wp.tile([128, 3, NF, 2, 128], FP8, name="w28sb", tag="w28sb")
            nc.gpsimd.dma_start(out=w28, in_=w28_d)
            w2t = wp.tile([128, 3, NT, 3, 128], BF16, name="w2sb", tag="w2sb")
            nc.gpsimd.dma_start(out=w2t, in_=w2_d)
            for hr, slot in ((0, (-2) % 4), (1, (-1) % 4)):
                conv_row(w1t, w18, xb, x8, slot, y1_epilogue(slot))
                nc.vector.tensor_scalar_mul(y1[slot], y1[slot], hm[:, hr:hr + 1])
                for coc in range(2):
                    # re-derive the fp8 copy from the masked bf16 halo row
                    nc.scalar.activation(
                        y18[slot][:, coc, PAD:PAD + W],
                        y1[slot][:, coc, PAD:PAD + W], AF.Copy, scale=SX)
                if hr == 0:
                    load_x_row(4, 0)      # x[0] -> slot 0

            # ---- main loop: fully unrolled 64 rows (ring slots = j%4) ----
            # Output stores lag their row by 2 so the store never gates the
            # next row's compute; ys slot k+2 holds row k (slots 0,1 dummy).
            y2s_tiles = {}
            for j in range(HS):
                y2s = op.tile([128, 3, W], BF16, name=f"y2s{j}", tag="y2s")
                y2s_tiles[j] = y2s
                # row j; prefetch x[j+1] (xs row j+5)
                load_x_row(j + 5, (j + 1) % 4)
                conv_row(w1t, w18, xb, x8, j % 4, y1_epilogue(j % 4))

                def y2_epilogue(coc, ps, j=j, y2s=y2s):
                    nc.scalar.activation(
                        y2s[:, coc, :], ps, AF.Prelu,
                        bias=b2c[:, coc:coc + 1], scale=1.0 / SB,
                        alpha=a2c[:, coc:coc + 1])
                    nc.vector.tensor_add(
                        y2s[:, coc, :], y2s[:, coc, :],
                        xb[j % 4][:, coc, PAD:PAD + W])

                conv_row(w2t, w28, y1, y18, j % 4, y2_epilogue,
                         dh0_last=True)
                if j >= 2:
                    nc.sync.dma_start(out=ys_d[:, ds(j * 3, 3), :],
                                      in_=y2s_tiles[j - 2])
            for j in (HS - 2, HS - 1):
                nc.sync.dma_start(out=ys_d[:, ds((j + 2) * 3, 3), :],
                                  in_=y2s_tiles[j])

    nc.compile()
    return nc


_NC_CACHE = {}


def _get_nc():
    if "nc" not in _NC_CACHE:
        _NC_CACHE["nc"] = _build_nc()
    return _NC_CACHE["nc"]


def kernel(x, w1, b1, a1, w2, b2, a2, _trace_dir=None, _trace_cores=None):
    x = np.asarray(x, np.float32)
    mask = _build_mask()
    w1m = np.asarray(w1, np.float32) * mask
    w2m = np.asarray(w2, np.float32) * mask

    # bf16 lhsT layout: [ci_mod(p), co_chunk, tap, ci_chunk, co_mod], *SB
    def wT(wm):
        wr = (wm * SB).reshape(3, 128, 3, 128, KS, KS)  # [oc, om, c, p, kh, kw]
        out = np.empty((128, 3, NT, 3, 128), np.float32)
        for t, (kh, kw) in enumerate(TAPS):
            out[:, :, t, :, :] = wr[:, :, :, :, kh, kw].transpose(3, 0, 2, 1)
        return np.ascontiguousarray(out.astype(BF16NP))

    # fp8 lhsT layout: [ci_mod(p), co_chunk, fp8_tap, ci_chunk(2), co_mod], *SW
    def wT8(wm):
        wr = (wm * SW).reshape(3, 128, 3, 128, KS, KS)
        out = np.empty((128, 3, NF, 2, 128), np.float32)
        for t, (kh, kw) in enumerate(FP8_TAPS):
            out[:, :, t, :, :] = wr[:, :, 0:2, :, kh, kw].transpose(3, 0, 2, 1)
        return np.ascontiguousarray(out.astype(E4NP))

    w1t_np, w2t_np = wT(w1m), wT(w2m)
    w18_np, w28_np = wT8(w1m), wT8(w2m)

    def chunked(v):  # [384] -> [128, 3]
        return np.ascontiguousarray(np.asarray(v, np.float32).reshape(3, 128).T)

    b1c, a1c = chunked(b1), chunked(a1)
    b2c, a2c = chunked(b2), chunked(a2)
    b1c16 = np.ascontiguousarray(b1c * np.float32(SX))

    xq = x.reshape(B, 3, 128, H, W)
    in_maps = []
    for core in range(NCORES):
        b_, s = divmod(core, SPB)
        r0 = s * HS
        xs = np.zeros((128, NR, 3, WP), BF16NP)
        lo, hi = r0 - 4, r0 - 4 + NR          # global rows [lo, hi)
        glo, ghi = max(lo, 0), min(hi, H)
        if ghi > glo:
            xs[:, glo - lo:ghi - lo, :, PAD:PAD + W] = \
                xq[b_, :, :, glo:ghi, :].transpose(1, 2, 0, 3)
        hmv = np.zeros((128, 2), np.float32) if s == 0 else np.ones((128, 2), np.float32)
        in_maps.append({
            "xs": xs.reshape(128, NR * 3, WP),
            "w1t": w1t_np, "w2t": w2t_np,
            "w18": w18_np, "w28": w28_np,
            "b1c": b1c, "a1c": a1c, "b2c": b2c, "a2c": a2c,
            "b1c16": b1c16,
            "hm": hmv,
        })

    nc = _get_nc()
    kw = {}
    if _trace_dir is not None:
        kw = dict(trace=True, tmpdir=_trace_dir,
                  trace_cores=_trace_cores or [0])

    def gather(res):
        y = np.empty_like(x)
        for core in range(NCORES):
            b_, s = divmod(core, SPB)
            r0 = s * HS
            ys = res.results[core]["ys"].reshape(128, HS + 2, 3, W)[:, 2:]
            y[b_, :, r0:r0 + HS, :] = \
                ys.transpose(2, 0, 1, 3).reshape(C, HS, W).astype(np.float32)
        return y

    res = y = None
    for attempt in range(4):
        try:
            res = run_bass_kernel_spmd(nc, in_maps,
                                       core_ids=list(range(NCORES)), **kw)
            y = gather(res)
            # transient device DMA failures ("DMA engine queue invalid") can
            # silently corrupt a run; expected |y|max ~ 12
            if np.isfinite(y).all() and np.abs(y).max() < 50.0:
                break
            if attempt == 3:
                break
        except Exception:
            # transient NRT/axon device errors recover on retry
            if attempt == 3:
                raise
            import time
            time.sleep(5)

    if _trace_dir is not None:
        return y, res
    return y


# revision 17
# speedup vs baseline: 1.0042x; 1.0042x over previous
"""Trainium2 Bass kernel for the EntropyResidualBlock (two masked 5x5 convs,
PReLU, residual) on 8 NeuronCores.

Sharding: 8 cores = 2 batches x 4 H-strips of 64 rows. Each core recomputes a
2-row y1 halo at the top of its strip (no cross-core communication); x halos
are host-zero-padded and a per-core halo-mask input zeroes the y1 halo rows
for strips at the image top (so conv2 sees correct zero padding).

The PixelCNN mask leaves only 13 of 25 taps nonzero, so each conv row is 13
taps x 3 ci-chunks x 3 co-chunks of [128,128]x[128,512] matmuls accumulated
in PSUM. To beat the bf16 PE roofline, 5 taps' (cic0,cic1) chunk pairs run as
fp8e4 DoubleRow matmuls (K=256 in one 512-cycle pass, measured 2.0x); the
rest stay bf16. Scales: w8 = e4m3(w*1024), x8/y18 = e4m3(16*act), bf16
weights *16384 so every matmul accumulates 16384*true in PSUM; the PReLU
epilogue applies scale=1/16384. Error budget (vs 2e-2 gate): ~1.9e-2,
verified against the exact reference inputs in simulation.

conv1 -> conv2 are fused through rolling 4-row SBUF ring buffers (bf16 + fp8
copies of each activation row).
"""

import os
import sys

import numpy as np
import ml_dtypes

for _p in ("/opt/trn_rl_repo",):
    if os.path.isdir(_p) and _p not in sys.path:
        sys.path.append(_p)

import concourse.bass as bass  # noqa: E402
import concourse.tile as tile  # noqa: E402
from concourse import bacc, mybir  # noqa: E402
from concourse.bass import ds  # noqa: E402
from concourse.bass_utils import run_bass_kernel_spmd  # noqa: E402

BF16NP = ml_dtypes.bfloat16
E4NP = ml_dtypes.float8_e4m3
F32 = mybir.dt.float32
BF16 = mybir.dt.bfloat16
FP8 = mybir.dt.float8e4
AF = mybir.ActivationFunctionType
DRM = mybir.MatmulPerfMode.DoubleRow

B, C, H, W = 2, 384, 256, 512
NG, CPN, KS, PAD = 16, 24, 5, 2
NCORES = 8
SPB = 4            # strips per batch
HS = H // SPB      # 64 output rows per core
WP = 520           # padded row width in SBUF/DRAM (2 left pad + 512 + 6)
WP8 = 528          # fp8 ring row pitch (mult of 16 for DoubleRow pair step)
NR = HS + 5        # x rows staged per core: r0-4 .. r0+64 (last is prefetch slack)
TAPS = [(kh, kw) for kh in (0, 1) for kw in range(KS)] + [(2, 0), (2, 1), (2, 2)]
NT = len(TAPS)     # 13
FP8_TAPS = [(0, 0), (0, 2), (0, 4), (1, 1), (1, 3)]
NF = len(FP8_TAPS)
SW, SX = 1024.0, 16.0      # fp8 scales (pow2; max|w|*SW~111<240, 16*act<240)
SB = SW * SX               # bf16 weight scale; PSUM holds SB * true


def _build_mask() -> np.ndarray:
    m = np.zeros((C, C, KS, KS), np.float32)
    m[:, :, :PAD, :] = 1.0
    m[:, :, PAD, :PAD] = 1.0
    g = np.arange(C) // CPN
    m[:, :, PAD, PAD] = (g[None, :] <= g[:, None]).astype(np.float32)
    return m


def _build_nc():
    nc = bacc.Bacc("TRN2", target_bir_lowering=False, debug=False,
                   num_devices=NCORES)
    xs_d = nc.dram_tensor("xs", [128, NR * 3, WP], BF16, kind="ExternalInput").ap()
    w1_d = nc.dram_tensor("w1t", [128, 3, NT, 3, 128], BF16,
                          kind="ExternalInput").ap()
    w2_d = nc.dram_tensor("w2t", [128, 3, NT, 3, 128], BF16,
                          kind="ExternalInput").ap()
    w18_d = nc.dram_tensor("w18", [128, 3, NF, 2, 128], FP8,
                           kind="ExternalInput").ap()
    w28_d = nc.dram_tensor("w28", [128, 3, NF, 2, 128], FP8,
                           kind="ExternalInput").ap()
    b1_d = nc.dram_tensor("b1c", [128, 3], F32, kind="ExternalInput").ap()
    a1_d = nc.dram_tensor("a1c", [128, 3], F32, kind="ExternalInput").ap()
    b2_d = nc.dram_tensor("b2c", [128, 3], F32, kind="ExternalInput").ap()
    a2_d = nc.dram_tensor("a2c", [128, 3], F32, kind="ExternalInput").ap()
    b116_d = nc.dram_tensor("b1c16", [128, 3], F32, kind="ExternalInput").ap()
    hm_d = nc.dram_tensor("hm", [128, 2], F32, kind="ExternalInput").ap()
    # rows 0,1 are dummies (garbage from the 2-row store lag); host reads 2..65
    ys_d = nc.dram_tensor("ys", [128, (HS + 2) * 3, W], BF16,
                          kind="ExternalOutput").ap()

    with tile.TileContext(nc) as tc:
        with tc.tile_pool(name="wp", bufs=1) as wp, \
             tc.tile_pool(name="cp", bufs=1) as cp, \
             tc.tile_pool(name="ring", bufs=1) as rp, \
             tc.tile_pool(name="op", bufs=4) as op, \
             tc.tile_pool(name="pp", bufs=8, space="PSUM") as pp:

            w18 = wp.tile([128, 3, NF, 2, 128], FP8, name="w18sb", tag="w18sb")
            nc.gpsimd.dma_start(out=w18, in_=w18_d)
            w1t = wp.tile([128, 3, NT, 3, 128], BF16, name="w1sb", tag="w1sb")
            # coc0 arrives per-tap so the very first matmuls aren't gated on
            # a bulk transfer; later chunks can be coarse
            for _t in range(NT):
                nc.gpsimd.dma_start(out=w1t[:, 0, _t], in_=w1_d[:, 0, _t])
            for _c in (1, 2):
                nc.gpsimd.dma_start(out=w1t[:, _c], in_=w1_d[:, _c])
            b1c = cp.tile([128, 3], F32, name="b1sb", tag="b1sb")
            nc.gpsimd.dma_start(out=b1c, in_=b1_d)
            a1c = cp.tile([128, 3], F32, name="a1sb", tag="a1sb")
            nc.gpsimd.dma_start(out=a1c, in_=a1_d)
            b2c = cp.tile([128, 3], F32, name="b2sb", tag="b2sb")
            nc.gpsimd.dma_start(out=b2c, in_=b2_d)
            a2c = cp.tile([128, 3], F32, name="a2sb", tag="a2sb")
            nc.gpsimd.dma_start(out=a2c, in_=a2_d)
            b1c16 = cp.tile([128, 3], F32, name="b116sb", tag="b116sb")
            nc.gpsimd.dma_start(out=b1c16, in_=b116_d)
            hm = cp.tile([128, 2], F32, name="hmsb", tag="hmsb")
            nc.gpsimd.dma_start(out=hm, in_=hm_d)

            xb = [rp.tile([128, 3, WP], BF16, name=f"xb{j}", tag=f"xb{j}")
                  for j in range(4)]
            x8 = [rp.tile([128, 2, WP8], FP8, name=f"x8{j}", tag=f"x8{j}")
                  for j in range(4)]
            y1 = [rp.tile([128, 3, WP], BF16, name=f"y1r{j}", tag=f"y1r{j}")
                  for j in range(4)]
            y18 = [rp.tile([128, 2, WP8], FP8, name=f"y18r{j}", tag=f"y18r{j}")
                   for j in range(4)]
            for j in range(4):
                # W-pad columns of the conv2-input rings stay zero forever
                nc.vector.memset(y1[j][:, :, 0:PAD], 0.0)
                nc.vector.memset(y1[j][:, :, PAD + W:WP], 0.0)
                nc.vector.memset(y18[j][:, :, 0:PAD], 0.0)
                nc.vector.memset(y18[j][:, :, PAD + W:WP8], 0.0)

            def load_x_row(row_expr, slot):
                # one DMA: [128, 3, WP] bf16 row (host zero-padded), + fp8 cast
                nc.sync.dma_start(out=xb[slot], in_=xs_d[:, ds(row_expr * 3, 3), :])
                nc.scalar.activation(x8[slot][:, :, 0:WP], xb[slot][:, 0:2, :],
                                     AF.Copy, scale=SX)

            def conv_row(wt, w8t, ring, ring8, hmod, epilogue, dh0_last=False):
                taps = sorted(TAPS, key=lambda t: t[0] == 2) if dh0_last else TAPS
                for coc in range(3):
                    ps = pp.tile([128, W], F32, name="ps", tag="ps")
                    # fp8 DoubleRow block first (mode switches only at block
                    # edges), then the bf16 block
                    for n8, (kh, kw) in enumerate(FP8_TAPS):
                        dh, dw = kh - 2, kw - 2
                        src8 = ring8[(hmod + dh) % 4]
                        fpi = FP8_TAPS.index((kh, kw))
                        nc.tensor.matmul(
                            ps,
                            lhsT=w8t[:, coc, fpi],
                            rhs=src8[:, 0:2, PAD + dw: PAD + dw + W],
                            start=(n8 == 0), stop=False, perf_mode=DRM)
                    work = [(kh, kw, cic) for (kh, kw) in taps for cic in range(3)
                            # fp8 pairs covered above; group-causal center tap:
                            # ci chunk 2 never feeds co chunk 0 - weights all 0
                            if not ((kh, kw) in FP8_TAPS and cic < 2)
                            and not (kh == 2 and kw == 2 and cic == 2 and coc == 0)]
                    for n, (kh, kw, cic) in enumerate(work):
                        dh, dw = kh - 2, kw - 2
                        src = ring[(hmod + dh) % 4]
                        ti = TAPS.index((kh, kw))
                        nc.tensor.matmul(
                            ps,
                            lhsT=wt[:, coc, ti, cic, :],
                            rhs=src[:, cic, PAD + dw: PAD + dw + W],
                            start=False, stop=(n == len(work) - 1))
                    epilogue(coc, ps)

            def y1_epilogue(slot):
                def f(coc, ps):
                    nc.scalar.activation(
                        y1[slot][:, coc, PAD:PAD + W], ps, AF.Prelu,
                        bias=b1c[:, coc:coc + 1], scale=1.0 / SB,
                        alpha=a1c[:, coc:coc + 1])
                    if coc < 2:
                        # fp8 copy of y1 (x16) for conv2's DoubleRow taps
                        nc.scalar.activation(
                            y18[slot][:, coc, PAD:PAD + W], ps, AF.Prelu,
                            bias=b1c16[:, coc:coc + 1], scale=SX / SB,
                            alpha=a1c[:, coc:coc + 1])
                return f

            # ---- prologue: x[-4..-1], then y1[-2], y1[-1] (halo, maskable) ----
            for j in range(4):
                load_x_row(j, j)          # xs row j = x[r0-4+j] -> slot j
            w28 = wp.tile([128, 3, NF, 2, 128], FP8, name="w28sb", tag="w28sb")
            nc.gpsimd.dma_start(out=w28, in_=w28_d)
            w2t = wp.tile([128, 3, NT, 3, 128], BF16, name="w2sb", tag="w2sb")
            nc.gpsimd.dma_start(out=w2t, in_=w2_d)
            for hr, slot in ((0, (-2) % 4), (1, (-1) % 4)):
                conv_row(w1t, w18, xb, x8, slot, y1_epilogue(slot))
                nc.vector.tensor_scalar_mul(y1[slot], y1[slot], hm[:, hr:hr + 1])
                for coc in range(2):
                    # re-derive the fp8 copy from the masked bf16 halo row
                    nc.scalar.activation(
                        y18[slot][:, coc, PAD:PAD + W],
                        y1[slot][:, coc, PAD:PAD + W], AF.Copy, scale=SX)
                if hr == 0:
                    load_x_row(4, 0)      # x[0] -> slot 0

            # ---- main loop: fully unrolled 64 rows (ring slots = j%4) ----
            # Output stores lag their row by 2 so the store never gates the
            # next row's compute; ys slot k+2 holds row k (slots 0,1 dummy).
            y2s_tiles = {}
            for j in range(HS):
                y2s = op.tile([128, 3, W], BF16, name=f"y2s{j}", tag="y2s")
                y2s_tiles[j] = y2s
                # row j; prefetch x[j+1] (xs row j+5)
                load_x_row(j + 5, (j + 1) % 4)
                conv_row(w1t, w18, xb, x8, j % 4, y1_epilogue(j % 4))

                def y2_epilogue(coc, ps, j=j, y2s=y2s):
                    nc.scalar.activation(
                        y2s[:, coc, :], ps, AF.Prelu,
                        bias=b2c[:, coc:coc + 1], scale=1.0 / SB,
                        alpha=a2c[:, coc:coc + 1])
                    nc.vector.tensor_add(
                        y2s[:, coc, :], y2s[:, coc, :],
                        xb[j % 4][:, coc, PAD:PAD + W])

                conv_row(w2t, w28, y1, y18, j % 4, y2_epilogue,
                         dh0_last=True)
                if j >= 2:
                    nc.sync.dma_start(out=ys_d[:, ds(j * 3, 3), :],
                                      in_=y2s_tiles[j - 2])
            for j in (HS - 2, HS - 1):
                nc.sync.dma_start(out=ys_d[:, ds((j + 2) * 3, 3), :],
                                  in_=y2s_tiles[j])

    nc.compile()
    return nc


_NC_CACHE = {}


def _get_nc():
    if "nc" not in _NC_CACHE:
        _NC_CACHE["nc"] = _build_nc()
    return _NC_CACHE["nc"]


def kernel(x, w1, b1, a1, w2, b2, a2, _trace_dir=None, _trace_cores=None):
    x = np.asarray(x, np.float32)
    mask = _build_mask()
    w1m = np.asarray(w1, np.float32) * mask
    w2m = np.asarray(w2, np.float32) * mask

    # bf16 lhsT layout: [ci_mod(p), co_chunk, tap, ci_chunk, co_mod], *SB
    def wT(wm):
        wr = (wm * SB).reshape(3, 128, 3, 128, KS, KS)  # [oc, om, c, p, kh, kw]
        out = np.empty((128, 3, NT, 3, 128), np.float32)
        for t, (kh, kw) in enumerate(TAPS):
            out[:, :, t, :, :] = wr[:, :, :, :, kh, kw].transpose(3, 0, 2, 1)
        return np.ascontiguousarray(out.astype(BF16NP))

    # fp8 lhsT layout: [ci_mod(p), co_chunk, fp8_tap, ci_chunk(2), co_mod], *SW
    def wT8(wm):
        wr = (wm * SW).reshape(3, 128, 3, 128, KS, KS)
        out = np.empty((128, 3, NF, 2, 128), np.float32)
        for t, (kh, kw) in enumerate(FP8_TAPS):
            out[:, :, t, :, :] = wr[:, :, 0:2, :, kh, kw].transpose(3, 0, 2, 1)
        return np.ascontiguousarray(out.astype(E4NP))

    w1t_np, w2t_np = wT(w1m), wT(w2m)
    w18_np, w28_np = wT8(w1m), wT8(w2m)

    def chunked(v):  # [384] -> [128, 3]
        return np.ascontiguousarray(np.asarray(v, np.float32).reshape(3, 128).T)

    b1c, a1c = chunked(b1), chunked(a1)
    b2c, a2c = chunked(b2), chunked(a2)
    b1c16 = np.ascontiguousarray(b1c * np.float32(SX))

    xq = x.reshape(B, 3, 128, H, W)
    in_maps = []
    for core in range(NCORES):
        b_, s = divmod(core, SPB)
        r0 = s * HS
        xs = np.zeros((128, NR, 3, WP), BF16NP)
        lo, hi = r0 - 4, r0 - 4 + NR          # global rows [lo, hi)
        glo, ghi = max(lo, 0), min(hi, H)
        if ghi > glo:
            xs[:, glo - lo:ghi - lo, :, PAD:PAD + W] = \
                xq[b_, :, :, glo:ghi, :].transpose(1, 2, 0, 3)
        hmv = np.zeros((128, 2), np.float32) if s == 0 else np.ones((128, 2), np.float32)
        in_maps.append({
            "xs": xs.reshape(128, NR * 3, WP),
            "w1t": w1t_np, "w2t": w2t_np,
            "w18": w18_np, "w28": w28_np,
            "b1c": b1c, "a1c": a1c, "b2c": b2c, "a2c": a2c,
            "b1c16": b1c16,
            "hm": hmv,
        })

    nc = _get_nc()
    kw = {}
    if _trace_dir is not None:
        kw = dict(trace=True, tmpdir=_trace_dir,
                  trace_cores=_trace_cores or [0])

    def gather(res):
        y = np.empty_like(x)
        for core in range(NCORES):
            b_, s = divmod(core, SPB)
            r0 = s * HS
            ys = res.results[core]["ys"].reshape(128, HS + 2, 3, W)[:, 2:]
            y[b_, :, r0:r0 + HS, :] = \
                ys.transpose(2, 0, 1, 3).reshape(C, HS, W).astype(np.float32)
        return y

    res = y = None
    for attempt in range(4):
        try:
            res = run_bass_kernel_spmd(nc, in_maps,
                                       core_ids=list(range(NCORES)), **kw)
            y = gather(res)
            # transient device DMA failures ("DMA engine queue invalid") can
            # silently corrupt a run; expected |y|max ~ 12
            if np.isfinite(y).all() and np.abs(y).max() < 50.0:
                break
            if attempt == 3:
                break
        except Exception:
            # transient NRT/axon device errors recover on retry
            if attempt == 3:
                raise
            import time
            time.sleep(5)

    if _trace_dir is not None:
        return y, res
    return y
